# revision 12
# baseline (speedup 1.0000x reference)
"""DTW (dynamic time warping) distance kernel for Trainium2, 8-core SPMD.

Problem: B=32 independent (x[b] in R^{1024x64}, y[b] in R^{1024x64}) pairs.
For each pair: dist = cdist(x, y) (euclidean, [1024, 1024]); DTW dynamic
program over dist; output D[N, M] scalar per pair.

Sharding: embarrassingly parallel over batch. 8 cores x 4 batches each.

Per-core algorithm:
  Phase 1 (cdist): dist^2 = xsq_i + ysq_j - 2 x.y^T via one augmented
  matmul per [128, 512] tile (K=65: 64 feature rows of -2*x^T plus a ones
  row pairing with a ysq row); xsq added as the ACT bias of the Relu pass;
  then Sqrt. Tiles are DMAed to one DRAM buffer in 32x32-tile-blocked
  layout dist[b][I][J][r][t] (i = 32I + r, j = 32J + t).

  Phase 2 (DTW): tile-wavefront DP on custom fused DVE instructions. The
  [32, 32] tile grid runs in skewed slots s = 2I + J (94 slots); partition
  p = 32b + I owns tile row I. A new slot starts every 16 row-steps so two
  slots are in flight. In steady state ONE pair instruction
  (DTW_FUSED_PAIR_ANT) advances BOTH slots by a row — its 66-element
  stream interleaves the two chains on even/odd elements, each computing
      out[0] = L_r,  out[t] = c[t] + min(W[t-1], W[t], out[t-1])
  where W = [L_{r-1}, X_{r-1}] is that chain's previous row (pitch 33)
  and c is its ring cost row. Boundary steps (a slot at row 0, or a lone
  slot) use the single-row op (DTW_FUSED_ROW_ANT). The left boundary L_r
  rides the ring's col r*33 (written from the west slot's X column by one
  strided copy per 16 rows); the top boundary (row 0) reads TOPf: the
  north tile's bottom row moved down one partition via stream_shuffle,
  garbage lanes forced to BIG. Inactive lanes stay BIG automatically
  (W memset BIG, ring costs 0, min(BIG, BIG) + c = BIG).
"""

import numpy as np

import concourse.bass as bass
import concourse.bacc as bacc
import concourse.mybir as mybir
from concourse.tile import TileContext
from concourse.masks import make_identity
from concourse import bass_utils

f32 = mybir.dt.float32
ADD = mybir.AluOpType.add
MIN = mybir.AluOpType.min
MAX = mybir.AluOpType.max
MULT = mybir.AluOpType.mult
ACT = mybir.ActivationFunctionType

N_CORES = 8
NB = 4          # batches per core
N = 1024        # rows (x length)
M = 1024        # cols (y length)
F = 64          # features
T = 32          # DP tile edge
G = 32          # tile grid edge (G*T == N == M)
BIG = 3.0e38    # finite stand-in for +inf
SHIFT1 = [0] + list(range(31))  # stream_shuffle: out[m] = in[m-1] per 32-block

BSZ = G * G * T * T       # dist elements per batch (1 Mi)
ISZ = G * T * T           # dist elements per tile row I (32 Ki)

# --------------------------------------------------------------------------
# Custom fused DVE ops. Per DP row, one instruction computes
#   out[0] = c[0] (the caller puts the left boundary L there),
#   out[t] = c[t] + min(w[t-1], w[t], out[t-1])   t = 1..32.
# The DSL can't express a two-op (min-then-add) fold, so the uOp programs
# are hand-built (the sanctioned escape hatch per 04-custom-dve-api.md),
# mirroring the stock tensor_tensor_scan's backward-routed A-flop feedback.
#
# uOp datapath (v3/TRN2, fp32 1x):
#   inputs: inp0 = w (SRC_0, feeds blk0), inp1 = c (SRC_1 -> delay lane 0)
#   blk0: BYPASS(inp0)                      -> raw w_t
#   blk1: MIN(prev, NEXT_A = blk2 A-flop)   -> m_t = min(w_t, w_{t-1});
#         lane1 <- blk0 out (raw w_t)
#   blk2: BYPASS(lane1) + A-flop <- w_t; lane2 <- blk1 out (m_t)
#   blk3: MIN(lane2, NEXT_A = blk4 A-flop)  -> min(m_t, out[t-1])
#   blk4: ADD(prev, lane0 = c_t) + A-flop <- out[t]
#   blk5..7: BYPASS relay -> WR0_LO
# Single-row op FSM: seed (A-flops <- +inf, ZERO) -> gap -> elem0 (blk1
# reads the +inf flop instead of w_0, so out[0] = 0 + c[0]) -> [bubble <->
# work] at 1 elem / 2 cycles until SRC_TENSOR_DONE. Constants enter only in
# the seed uop: a per-partition s0 AP would add ~120ns to every instruction
# (measured), so s0 stays an unused immediate.
# --------------------------------------------------------------------------

from concourse.dve_uop import (
    AluInp,
    AluOp as UAluOp,
    DelayInp,
    DveOpSpec,
    ENABLE,
    InpSel,
    OutPath,
    OutSel,
    Trigger,
    UopConfig,
)
from concourse.dve_spec import Spec, Src0, Src1, C0, minn
from concourse import dve_ops as _dvo

_NAME = "DTW_FUSED_ROW_ANT"

_L_C = 0   # lane 0: cost stream (from inp1)
_L_W = 1   # lane 1: raw w relay blk1 -> blk2
_L_M = 2   # lane 2: window-min relay blk2 -> blk3

_COUNT1 = (Trigger.COUNT, Trigger.NONE, Trigger.NONE)


def _work_uop(elem0: bool) -> UopConfig:
    u = UopConfig()
    u.enable_input(InpSel.SRC_0, 0)
    u.enable_input(InpSel.SRC_1, 1)
    dp = u.datapath_config
    # c rides lane 0: loaded from inp1 at blk0, passed through blk3.
    for k in range(4):
        dp[k].pass_through_delay(_L_C)
    dp[0].enable_alu(UAluOp.BYPASS, AluInp.PREV_ALU_OUT, AluInp.PREV_ALU_OUT)
    if elem0:
        dp[1].enable_alu(UAluOp.BYPASS, AluInp.NEXT_ALU_OUT_A, AluInp.NEXT_ALU_OUT_A)
    else:
        dp[1].enable_alu(UAluOp.MIN, AluInp.PREV_ALU_OUT, AluInp.NEXT_ALU_OUT_A)
    dp[1].enable_delay_from_src(DelayInp.PREV_ALU_OUT, _L_W)
    dp[2].enable_alu(UAluOp.BYPASS, AluInp.PREV_DELAY_1, AluInp.PREV_DELAY_1)
    dp[2].alu_out_a_enable = ENABLE
    dp[2].enable_delay_from_src(DelayInp.PREV_ALU_OUT, _L_M)
    dp[3].enable_alu(UAluOp.MIN, AluInp.PREV_DELAY_2, AluInp.NEXT_ALU_OUT_A)
    dp[4].enable_alu(UAluOp.ADD, AluInp.PREV_ALU_OUT, AluInp.PREV_DELAY_0)
    dp[4].alu_out_a_enable = ENABLE
    for k in range(5, 8):
        dp[k].enable_alu(UAluOp.BYPASS, AluInp.PREV_ALU_OUT, AluInp.PREV_ALU_OUT)
    u.require_inp0 = ENABLE
    u.require_inp1 = ENABLE
    u.enable_output(OutSel.ALU_OUT, OutPath.WR0_LO)
    u.repeat_count = 1
    return u


def _bubble_uop(nxt: int) -> UopConfig:
    # stock-scan style: a fully inert cycle (no inputs, no ALUs, no writes).
    u = UopConfig()
    u.trigger = _COUNT1
    u.repeat_count = 1
    u.next_uop = (nxt, 0, 0)
    return u


def _dtw_row_uops() -> list[UopConfig]:
    # uop[0]: seed. inp0 = +inf (bypassed down the out-flop chain into
    # blk2's A-flop), inp1 = ZERO (rides lane 0 into blk4's A-flop), so
    # elem0 computes out[0] = min(+inf, 0) + c[0] = c[0], and the caller
    # places the left boundary L in the c stream's element 0.
    u0 = UopConfig()
    u0.enable_input(InpSel.POS_INF, 0)
    u0.enable_input(InpSel.ZERO, 1)
    dp = u0.datapath_config
    for k in range(4):
        dp[k].pass_through_delay(_L_C)
    dp[0].enable_alu(UAluOp.BYPASS, AluInp.PREV_ALU_OUT, AluInp.PREV_ALU_OUT)
    dp[1].enable_alu(UAluOp.BYPASS, AluInp.PREV_ALU_OUT, AluInp.PREV_ALU_OUT)
    dp[2].enable_alu(UAluOp.BYPASS, AluInp.PREV_ALU_OUT, AluInp.PREV_ALU_OUT)
    dp[2].alu_out_a_enable = ENABLE
    dp[3].enable_alu(UAluOp.BYPASS, AluInp.PREV_DELAY_0, AluInp.PREV_DELAY_0)
    dp[4].enable_alu(UAluOp.BYPASS, AluInp.PREV_ALU_OUT, AluInp.PREV_ALU_OUT)
    dp[4].alu_out_a_enable = ENABLE
    u0.trigger = _COUNT1
    u0.repeat_count = 1
    u0.next_uop = (1, 0, 0)

    u1 = _bubble_uop(2)  # gap: elem0 trails the seed's flop writes

    u2 = _work_uop(elem0=True)
    u2.trigger = _COUNT1
    u2.next_uop = (3, 0, 0)

    u3 = _bubble_uop(4)  # recurrence bubble (half rate)

    u4 = _work_uop(elem0=False)
    u4.trigger = (Trigger.SRC_TENSOR_DONE, Trigger.COUNT, Trigger.NONE)
    u4.next_uop = (0, 3, 0)

    return [u0, u1, u2, u3, u4]


class _DtwRowOp:
    """Shim with the DveOp interface whose uOp program is hand-built (the
    min-plus two-op fold is not DSL-expressible; hand-edited programs are
    the documented escape hatch in 04-custom-dve-api.md)."""

    name = _NAME
    subdim = False
    spec = Spec(body=minn(Src0, C0) + Src1)  # leaf-accurate stand-in

    def __init__(self):
        self._cache = {}

    def compile(self, ver):
        if ver not in self._cache:
            assert ver == "v3", f"{_NAME}: only TRN2/v3 uops are defined"
            s = DveOpSpec(
                name=self.name,
                opcode=_dvo.get_dve_sub_opcode(self.name),
                uops=_dtw_row_uops(),
                rd1_en=True,
            )
            s.validate(ver)
            self._cache[ver] = s
        return self._cache[ver]


def _register_op():
    for op in _dvo.OPS:
        if op.name == _NAME:
            return op
    if _NAME not in _dvo._SUB_OPCODE_FOR_NAME:
        row = max(_dvo._SUB_OPCODE_FOR_NAME.values()) + 1
        assert row < 0x20, "no free custom-DVE opcode rows"
        _dvo._SUB_OPCODE_FOR_NAME[_NAME] = row
    op = _DtwRowOp()
    _dvo.OPS.append(op)
    _dvo.CUSTOM_DVE_SPECS[_NAME] = op.spec
    return op


_DTW_OP = _register_op()

# --------------------------------------------------------------------------
# Pair op: ONE instruction advances BOTH in-flight slots by one row. The
# 66-element stream interleaves chain E (even-parity slot) on even elements
# and chain O (odd parity) on odd elements, so each chain's elements are
# naturally 2 cycles apart — full 1 elem/cycle rate, no bubble uops. Chain E
# uses the A-flops of blk2/blk4, chain O the B-flops (configs travel with
# elements, so alternating uop configs keep the two recurrences separate).
# --------------------------------------------------------------------------

_PAIR_NAME = "DTW_FUSED_PAIR_ANT"


def _pair_work_uop(odd: bool, elem0: bool) -> UopConfig:
    nxt = AluInp.NEXT_ALU_OUT_B if odd else AluInp.NEXT_ALU_OUT_A
    u = UopConfig()
    u.enable_input(InpSel.SRC_0, 0)
    u.enable_input(InpSel.SRC_1, 1)
    dp = u.datapath_config
    for k in range(4):
        dp[k].pass_through_delay(_L_C)
    dp[0].enable_alu(UAluOp.BYPASS, AluInp.PREV_ALU_OUT, AluInp.PREV_ALU_OUT)
    if elem0:
        dp[1].enable_alu(UAluOp.BYPASS, nxt, nxt)
    else:
        dp[1].enable_alu(UAluOp.MIN, AluInp.PREV_ALU_OUT, nxt)
    dp[1].enable_delay_from_src(DelayInp.PREV_ALU_OUT, _L_W)
    dp[2].enable_alu(UAluOp.BYPASS, AluInp.PREV_DELAY_1, AluInp.PREV_DELAY_1)
    dp[2].enable_delay_from_src(DelayInp.PREV_ALU_OUT, _L_M)
    dp[3].enable_alu(UAluOp.MIN, AluInp.PREV_DELAY_2, nxt)
    dp[4].enable_alu(UAluOp.ADD, AluInp.PREV_ALU_OUT, AluInp.PREV_DELAY_0)
    if odd:
        dp[2].alu_out_b_enable = ENABLE
        dp[4].alu_out_b_enable = ENABLE
    else:
        dp[2].alu_out_a_enable = ENABLE
        dp[4].alu_out_a_enable = ENABLE
    for k in range(5, 8):
        dp[k].enable_alu(UAluOp.BYPASS, AluInp.PREV_ALU_OUT, AluInp.PREV_ALU_OUT)
    u.require_inp0 = ENABLE
    u.require_inp1 = ENABLE
    u.enable_rev_ops = ENABLE
    u.enable_output(OutSel.ALU_OUT, OutPath.WR0_LO)
    u.repeat_count = 1
    return u


def _pair_seed_uop(odd: bool, nxt_idx: int) -> UopConfig:
    u = UopConfig()
    u.enable_input(InpSel.POS_INF, 0)
    u.enable_input(InpSel.ZERO, 1)
    dp = u.datapath_config
    for k in range(4):
        dp[k].pass_through_delay(_L_C)
    for k in (0, 1, 2, 4):
        dp[k].enable_alu(UAluOp.BYPASS, AluInp.PREV_ALU_OUT, AluInp.PREV_ALU_OUT)
    dp[3].enable_alu(UAluOp.BYPASS, AluInp.PREV_DELAY_0, AluInp.PREV_DELAY_0)
    if odd:
        dp[2].alu_out_b_enable = ENABLE
        dp[4].alu_out_b_enable = ENABLE
    else:
        dp[2].alu_out_a_enable = ENABLE
        dp[4].alu_out_a_enable = ENABLE
    u.trigger = _COUNT1
    u.repeat_count = 1
    u.next_uop = (nxt_idx, 0, 0)
    return u


def _dtw_pair_uops() -> list[UopConfig]:
    u0 = _pair_seed_uop(odd=False, nxt_idx=1)
    u1 = _pair_seed_uop(odd=True, nxt_idx=2)
    u2 = _pair_work_uop(odd=False, elem0=True)
    u2.trigger = _COUNT1
    u2.next_uop = (3, 0, 0)
    u3 = _pair_work_uop(odd=True, elem0=True)
    u3.trigger = _COUNT1
    u3.next_uop = (4, 0, 0)
    u4 = _pair_work_uop(odd=False, elem0=False)
    u4.trigger = (Trigger.SRC_TENSOR_DONE, Trigger.COUNT, Trigger.NONE)
    u4.next_uop = (0, 5, 0)
    u5 = _pair_work_uop(odd=True, elem0=False)
    u5.trigger = (Trigger.SRC_TENSOR_DONE, Trigger.COUNT, Trigger.NONE)
    u5.next_uop = (0, 4, 0)
    return [u0, u1, u2, u3, u4, u5]


class _DtwPairOp:
    name = _PAIR_NAME
    subdim = False
    spec = Spec(body=minn(Src0, C0) + Src1)

    def __init__(self):
        self._cache = {}

    def compile(self, ver):
        if ver not in self._cache:
            assert ver == "v3", f"{_PAIR_NAME}: only TRN2/v3 uops are defined"
            s = DveOpSpec(
                name=self.name,
                opcode=_dvo.get_dve_sub_opcode(self.name),
                uops=_dtw_pair_uops(),
                rd1_en=True,
            )
            s.validate(ver)
            self._cache[ver] = s
        return self._cache[ver]


def _register_pair_op():
    for op in _dvo.OPS:
        if op.name == _PAIR_NAME:
            return op
    if _PAIR_NAME not in _dvo._SUB_OPCODE_FOR_NAME:
        row = max(_dvo._SUB_OPCODE_FOR_NAME.values()) + 1
        assert row < 0x20, "no free custom-DVE opcode rows"
        _dvo._SUB_OPCODE_FOR_NAME[_PAIR_NAME] = row
    op = _DtwPairOp()
    _dvo.OPS.append(op)
    _dvo.CUSTOM_DVE_SPECS[_PAIR_NAME] = op.spec
    return op


_DTW_PAIR_OP = _register_pair_op()
PAIR = True


def _emit_cdist(nc, sb, ps, psmm, x_d, y_d, dist_d, n_rows):
    """Emit phase 1. dist_d: DRAM [NB * BSZ], tile-blocked layout."""
    n_ti = n_rows // 128

    ident = sb.tile([128, 128], f32)
    make_identity(nc, ident[:])
    ones64 = sb.tile([64, 1], f32)
    nc.vector.memset(ones64[:], 1.0)

    XTA, YTA, XSQ = [], [], []
    deferred = []   # (b, XN, xta, xsq) — i-tiles 4..7 prep, emitted after ti=0
    for b in range(NB):
        XN = sb.tile([128, 8 * F], f32, tag="XN", bufs=4)
        YN = sb.tile([128, 8 * F], f32, tag="YN", bufs=2)
        xta = sb.tile([65, N], f32, tag=f"XTA{b}")
        yta = sb.tile([65, M], f32, tag=f"YTA{b}")
        xsq = sb.tile([128, 8], f32, tag=f"XSQ{b}")
        ysqel = sb.tile([64, M], f32, tag="YSQel", bufs=2)
        sqs = sb.tile([128, F], f32, tag="sqs", bufs=2)

        # natural-layout loads: partition = i%128, free = (i//128, f).
        nc.gpsimd.dma_start(
            XN[:], bass.AP(x_d, b * N * F, [[F, 128], [128 * F, 8], [1, F]])
        )
        nc.gpsimd.dma_start(
            YN[:], bass.AP(y_d, b * M * F, [[F, 128], [128 * F, 8], [1, F]])
        )

        # PE transposes -> feature-major; x scaled by -2 on the PSUM copy-out.
        for g in range(2):
            pt = ps.tile([64, 512], f32, tag="pt")
            for tt in range(4):
                t = 4 * g + tt
                nc.tensor.transpose(
                    pt[:, tt * 128 : (tt + 1) * 128],
                    YN[:, t * F : (t + 1) * F], ident[:],
                )
            nc.scalar.activation(yta[0:64, g * 512 : (g + 1) * 512], pt[:], ACT.Copy)
        for g in range(1):  # g=1 (i-tiles 4..7) deferred past ti=0 dist
            pt = ps.tile([64, 512], f32, tag="pt")
            nt = min(4, n_ti - 4 * g)
            for tt in range(nt):
                t = 4 * g + tt
                nc.tensor.transpose(
                    pt[:, tt * 128 : (tt + 1) * 128],
                    XN[:, t * F : (t + 1) * F], ident[:],
                )
            nc.scalar.activation(
                xta[0:64, g * 512 : g * 512 + nt * 128],
                pt[:, 0 : nt * 128], ACT.Copy, scale=-2.0,
            )
        # xsq[i] per i-tile column (ACT Square with accumulate)
        for t in range(4):
            nc.scalar.activation(
                sqs[:], XN[:, t * F : (t + 1) * F], ACT.Square,
                accum_out=xsq[:, t : t + 1],
            )
        deferred.append((b, XN, xta, xsq))
        # augmented rows: xta row 64 = ones; yta row 64 = ysq
        nc.vector.memset(xta[64:65, :], 1.0)
        nc.gpsimd.tensor_tensor(ysqel[:], yta[0:64, :], yta[0:64, :], MULT)
        for nj in range(2):
            py = ps.tile([1, 512], f32, tag="py")
            nc.tensor.matmul(
                py[:], ones64[:], ysqel[:, nj * 512 : (nj + 1) * 512],
                start=True, stop=True,
            )
            nc.scalar.activation(
                yta[64:65, nj * 512 : (nj + 1) * 512], py[:], ACT.Copy
            )
        XTA.append(xta)
        YTA.append(yta)
        XSQ.append(xsq)

    # dist tiles: matmul + relu(+xsq bias) + sqrt + DMA out tile-blocked.
    for ti in range(n_ti):
        if ti == 1:
            # late prep (i-tiles 4..7): off the slot-0 critical path
            for b, XN, xta, xsq in deferred:
                pt = ps.tile([64, 512], f32, tag="pt")
                for tt in range(4):
                    t = 4 + tt
                    nc.tensor.transpose(
                        pt[:, tt * 128 : (tt + 1) * 128],
                        XN[:, t * F : (t + 1) * F], ident[:],
                    )
                nc.scalar.activation(
                    xta[0:64, 512:1024], pt[:], ACT.Copy, scale=-2.0,
                )
                sqs = sb.tile([128, F], f32, tag="sqs", bufs=2)
                for t in range(4, n_ti):
                    nc.scalar.activation(
                        sqs[:], XN[:, t * F : (t + 1) * F], ACT.Square,
                        accum_out=xsq[:, t : t + 1],
                    )
        for b in range(NB):
            ds2 = sb.tile([128, 1024], f32, tag="DS2", bufs=2)
            for nj in range(2):
                pq = psmm.tile([128, 512], f32, tag="pq")
                nc.tensor.matmul(
                    pq[:],
                    XTA[b][:, ti * 128 : (ti + 1) * 128],
                    YTA[b][:, nj * 512 : (nj + 1) * 512],
                    start=True, stop=True,
                )
                ds = sb.tile([128, 512], f32, tag="DS", bufs=3)
                nc.scalar.activation(
                    ds[:], pq[:], ACT.Relu, bias=XSQ[b][:, ti : ti + 1]
                )
                nc.scalar.activation(
                    ds2[:, nj * 512 : (nj + 1) * 512], ds[:], ACT.Sqrt
                )
            # -> dist_d[b][I][J][r][t] with I = 4 ti + Ii, i_local = 32 Ii + r
            for Ii in range(4):
                dst = bass.AP(
                    dist_d, b * BSZ + (ti * 4 + Ii) * ISZ,
                    [[T, T], [T * T, G], [1, T]],
                )
                src = bass.AP(
                    ds2.tensor, Ii * 32 * 1024,
                    [[1024, T], [T, G], [1, T]],
                )
                nc.sync.dma_start(dst, src)


def _emit_dtw(nc, tc, sb, dist_d, xout_d):
    """Emit phase 2: skewed-slot wavefront, two slots in flight. Steady
    steps issue ONE pair op advancing both slots a row; boundary steps
    (a slot at row 0, or only one slot active) fall back to single fused
    ops. W parities and the 4 ring slots live in single merged tiles so a
    pair op's interleaved access pattern has constant strides."""
    NSLOT = 4
    RP = 33 * T                         # ring / W row pitch
    NS = 3 * G - 2                      # slots: s = 2I + J in [0, 93]
    LAG = 16                            # row-steps between slot starts
    RNG = sb.tile([128, NSLOT * RP], f32, name="rngall")
    Wall = sb.tile([128, 2 * RP], f32, name="wall")
    TOPr = [
        sb.tile([128, 33], f32, name=f"TOPr{k}") for k in range(2)
    ]
    TOPf = [
        sb.tile([128, 33], f32, name=f"TOPf{k}") for k in range(2)
    ]
    INJ = sb.tile([128, 1], f32)

    nc.vector.memset(RNG[:], 0.0)       # inactive lanes: costs >= 0
    nc.vector.memset(Wall[:], BIG)
    nc.vector.memset(INJ[:], -BIG)
    for b in range(NB):                 # I = 0 lanes: top boundary is BIG
        nc.vector.memset(INJ[32 * b : 32 * b + 1, :], BIG)

    def issue_dma(s):
        # diagonal load for slot s: partition 32b + I gets tile
        # (I, s - 2I); row r at RNG[:, (s%4)*RP + r*33 + 1 : +33] (col
        # r*33 holds the row's left boundary, written by copy_l). Issued
        # two slots early so the transfer is long done.
        if s >= NS:
            return
        ilo = max(0, (s - (G - 1) + 1) // 2)
        ihi = min(G - 1, s // 2)
        cnt = ihi - ilo + 1
        for b in range(NB):
            dst = bass.AP(
                RNG.tensor,
                (32 * b + ilo) * (NSLOT * RP) + (s % NSLOT) * RP + 1,
                [[NSLOT * RP, cnt], [33, T], [1, T]],
            )
            src = bass.AP(
                dist_d, b * BSZ + ilo * ISZ + (s - 2 * ilo) * T * T,
                [[ISZ - 2 * T * T, cnt], [T, T], [1, T]],
            )
            nc.gpsimd.dma_start(dst, src)

    def start_slot(s):
        p = s % 2
        # top boundary from slot s-2 (same parity) row 31, one partition
        # down; TOPr[32b] garbage -> forced BIG via INJ.
        nc.vector.stream_shuffle(
            TOPr[p][:], Wall[:, p * RP + 31 * 33 : p * RP + 31 * 33 + 33],
            SHIFT1,
        )
        nc.vector.scalar_tensor_tensor(
            TOPf[p][:], TOPr[p][:], INJ[:, 0:1], TOPr[p][:], MAX, MAX
        )
        if s == 0:
            for b in range(NB):         # D[0,0] corner
                nc.vector.memset(TOPf[p][32 * b : 32 * b + 1, 0:1], 0.0)

    def copy_l(s, r0):
        # left boundaries: L_r = X_r[31] of slot s-1 -> ring col r*33, for
        # rows r0..r0+15 (slot s-1 is >= 16 rows ahead). The fused op then
        # emits out[0] = 0 + c[0] = L_r (its state seeds to hardware ZERO),
        # keeping s0 an immediate — a per-partition s0 AP adds ~120ns/op.
        p = s % 2
        nc.vector.tensor_copy(
            bass.AP(RNG.tensor, (s % NSLOT) * RP + r0 * 33,
                    [[NSLOT * RP, 128], [33, LAG]]),
            bass.AP(Wall.tensor, (1 - p) * RP + r0 * 33 + 32,
                    [[2 * RP, 128], [33, LAG]]),
        )

    def row_fused(s, r):
        p = s % 2
        src = (
            TOPf[p][:] if r == 0
            else Wall[:, p * RP + (r - 1) * 33 : p * RP + r * 33]
        )
        nc.vector._custom_dve(
            _DTW_OP,
            out=Wall[:, p * RP + r * 33 : p * RP + r * 33 + 33],
            in0=src,
            in1=RNG[:, (s % NSLOT) * RP + r * 33 : (s % NSLOT) * RP + r * 33 + 33],
            s0=0.0,
        )

    def row_pair(active):
        # one instruction advances both slots: element order interleaves
        # chain E (even-parity slot) and chain O per stream position.
        (sA, rA), (sB, rB) = active
        (sE, rE), (sO, rO) = (
            ((sA, rA), (sB, rB)) if sA % 2 == 0 else ((sB, rB), (sA, rA))
        )
        dW = RP + (rO - rE) * 33
        dR = (sO % NSLOT - sE % NSLOT) * RP + (rO - rE) * 33
        in0 = bass.AP(
            Wall.tensor, (rE - 1) * 33, [[2 * RP, 128], [1, 33], [dW, 2]]
        )
        out = bass.AP(
            Wall.tensor, rE * 33, [[2 * RP, 128], [1, 33], [dW, 2]]
        )
        in1 = bass.AP(
            RNG.tensor, (sE % NSLOT) * RP + rE * 33,
            [[NSLOT * RP, 128], [1, 33], [dR, 2]],
        )
        nc.vector._custom_dve(_DTW_PAIR_OP, out=out, in0=in0, in1=in1, s0=0.0)

    issue_dma(0)
    issue_dma(1)
    active = []                         # (slot, next_row)
    t = 0
    done = 0
    # Logical per-macro-step timestamps order the wavefront for the Tile
    # scheduler.
    STEP = 0.0005                       # ms per macro-step (logical)
    while done < NS:
        if t % LAG == 0 and t // LAG < NS:
            s = t // LAG
            issue_dma(s + 2)
            with tc.tile_wait_until(t * STEP):
                start_slot(s)
                copy_l(s, 0)
            active.append([s, 0])
        with tc.tile_wait_until(t * STEP + STEP / 2):
            for a in active:
                if a[1] == LAG:     # slot s-1 just finished rows 16..31
                    copy_l(a[0], LAG)
            if PAIR and len(active) == 2 and all(a[1] >= 1 for a in active):
                row_pair(active)
                for a in active:
                    a[1] += 1
            else:
                for a in active:
                    row_fused(a[0], a[1])
                    a[1] += 1
        done += sum(1 for a in active if a[1] == T)
        active = [a for a in active if a[1] < T]
        t += 1

    nc.sync.dma_start(
        xout_d[:], Wall[:, (NS - 1) % 2 * RP + 31 * 33 + 32
                        : (NS - 1) % 2 * RP + 31 * 33 + 33]
    )


def build_nc(n_rows=N):
    nc = bacc.Bacc()
    x_d = nc.dram_tensor("x", [NB, N, F], f32, kind="ExternalInput")
    y_d = nc.dram_tensor("y", [NB, M, F], f32, kind="ExternalInput")
    xout_d = nc.dram_tensor("xout", [128, 1], f32, kind="ExternalOutput")

    with TileContext(nc) as tc:
        with (
            tc.tile_pool(name="sb", bufs=1) as sb,
            tc.tile_pool(name="ps", bufs=2, space="PSUM") as ps,
            tc.tile_pool(name="psmm", bufs=4, space="PSUM") as psmm,
            tc.tile_pool(name="dr", bufs=1, space="DRAM") as dr,
        ):
            dist_t = dr.tile([NB * BSZ], f32, name="distbuf")
            _emit_cdist(nc, sb, ps, psmm, x_d, y_d, dist_t.tensor, n_rows)
            _emit_dtw(nc, tc, sb, dist_t.tensor, xout_d)
    nc.compile()
    return nc


_NC_CACHE = {}


def _get_nc(n_rows=N):
    if n_rows not in _NC_CACHE:
        _NC_CACHE[n_rows] = build_nc(n_rows)
    return _NC_CACHE[n_rows]


def _make_in_maps(x, y):
    return [
        {"x": np.ascontiguousarray(x[NB * c : NB * (c + 1)]),
         "y": np.ascontiguousarray(y[NB * c : NB * (c + 1)])}
        for c in range(N_CORES)
    ]


def _extract_out(results):
    out = np.empty((N_CORES * NB,), np.float32)
    for c in range(N_CORES):
        xo = results[c]["xout"]
        for b in range(NB):
            out[NB * c + b] = xo[32 * b + 31, 0]
    return out


def kernel(x: np.ndarray, y: np.ndarray) -> np.ndarray:
    """x, y: [32, 1024, 64] float32 -> [32] float32 of DTW distances."""
    x = np.ascontiguousarray(x, dtype=np.float32)
    y = np.ascontiguousarray(y, dtype=np.float32)
    nc = _get_nc()
    res = bass_utils.run_bass_kernel_spmd(
        nc, _make_in_maps(x, y), core_ids=list(range(N_CORES))
    )
    return _extract_out(res.results)
